# revision 28
# baseline (speedup 1.0000x reference)
"""Trainium2 Bass kernel for nn_MessagePassingLayer (gnn_message_passing).

Computes, for x:[B,C,N,1] f32, edge_index:[B,N,K] i32, alpha scalar:
    out[b,c,n] = x[b,c,n]*(1+alpha) + sum_k x[b,c,edge_index[b,n,k]]

Sharding: B=8 batch samples, one per NeuronCore (data parallel). Edge
indices are intra-sample so there is no cross-core communication.

Per-core device program:
  - load node-major table xt [N=4096, C=64] (host-transposed layout)
  - 16x dma_gather (SWDGE row gather from HBM, 4096 rows x 256B each)
  - DVE pairwise-tree accumulation of the 16 gathered tiles
  - out = xt*(1+alpha) + sum  (DVE), stored node-major; host transposes back
"""
import os
import sys
import types

import numpy as np

B, C, N, K = 8, 64, 4096, 16
NCORES = 8
P = 128
COLS = N // P  # 32 nodes per partition
FREE = COLS * C  # 2048 f32 per partition

LAST_EXEC_NS = None


# ---------------------------------------------------------------------------
# axon NTFF profile hook shim (the agent image's antenv lacks axon_hooks)
# ---------------------------------------------------------------------------
def _install_profile_shim():
    if "antenv.axon_hooks" in sys.modules:
        return
    try:
        import antenv

        mod = types.ModuleType("antenv.axon_hooks")
        mod._hook = None
        mod.set_axon_ntff_profile_hook = lambda h: setattr(mod, "_hook", h)
        mod.get_axon_ntff_profile_hook = lambda: mod._hook
        sys.modules["antenv.axon_hooks"] = mod
        antenv.axon_hooks = mod
        from trn_agent_boot.trn_boot import _ntff_profile_via_ctypes

        mod.set_axon_ntff_profile_hook(
            _ntff_profile_via_ctypes("/opt/axon/libaxon_pjrt.so")
        )
    except Exception:
        pass


# ---------------------------------------------------------------------------
# Walrus in this container rejects >1 sync-wait per instruction. Split any
# multi-wait instruction into single-wait NoOps on the same engine.
# ---------------------------------------------------------------------------
def _split_multiwaits(nc, mybir):
    cnt = [0]
    for f in nc.m.functions:
        for bb in f.blocks:
            new_list = []
            for ins in bb.instructions:
                si = ins.sync_info
                if si is not None and si.on_wait and len(si.on_wait) > 1:
                    waits = list(si.on_wait)
                    for w in waits[:-1]:
                        cnt[0] += 1
                        nop = mybir.InstNoOp(name=f"I-waitsplit-{cnt[0]}")
                        nop.engine = ins.engine
                        nop.sync_info = mybir.SyncInfo(on_wait=[w], on_update=[])
                        try:
                            nc.register_instruction(nop, overwrite=True)
                        except Exception:
                            pass
                        new_list.append(nop)
                    ins.sync_info = mybir.SyncInfo(
                        on_wait=[waits[-1]], on_update=list(si.on_update)
                    )
                new_list.append(ins)
            bb.instructions = new_list


# ---------------------------------------------------------------------------
# Device program
# ---------------------------------------------------------------------------
GATHER_CHUNK = int(os.environ.get("KERNEL_GATHER_CHUNK", "1024"))
SCRATCH = int(os.environ.get("KERNEL_SCRATCH", "16384"))
SINGLE_PACKET = bool(int(os.environ.get("KERNEL_SINGLE_PACKET", "0")))


def _build_program():
    import concourse.mybir as mybir
    import concourse.tile as tile
    from concourse import bacc

    nc = bacc.Bacc("TRN2", target_bir_lowering=False, debug=False,
                   num_devices=NCORES, num_swdge_queues=4,
                   dynamic_dma_scratch_size=SCRATCH)
    xt_d = nc.dram_tensor("xt", [N, C], mybir.dt.float32, kind="ExternalInput")
    idx_d = nc.dram_tensor("idx", [P, K * (N // 16)], mybir.dt.int16,
                           kind="ExternalInput")
    alpha_d = nc.dram_tensor("alpha", [P, 1], mybir.dt.float32,
                             kind="ExternalInput")
    out_d = nc.dram_tensor("out", [N, C], mybir.dt.float32,
                           kind="ExternalOutput")

    cpk = N // GATHER_CHUNK      # segments (chunk instructions per k)
    ipg = GATHER_CHUNK // 16     # idx cols per chunk
    opg = GATHER_CHUNK // P      # out free-cols per chunk
    SEGW = opg * C               # f32 cols per segment in node-major tiles

    with tile.TileContext(nc) as tc:
        with tc.tile_pool(name="sbuf", bufs=1) as pool:
            xt_sb = pool.tile([P, FREE], mybir.dt.float32, tag="xt")
            idx_sb = [pool.tile([P, K * ipg], mybir.dt.int16, tag=f"idx{c}",
                                name=f"idx{c}") for c in range(cpk)]
            al_sb = pool.tile([P, 1], mybir.dt.float32, tag="al")
            g = [[pool.tile([P, SEGW], mybir.dt.float32, tag=f"g{k}_{c}",
                            name=f"g{k}_{c}") for c in range(cpk)]
                 for k in range(K)]
            o = [pool.tile([P, SEGW], mybir.dt.float32, tag=f"o{c}",
                           name=f"o{c}") for c in range(cpk)]

            xt_nm = xt_d.ap().rearrange("(p a) c -> p (a c)", p=P)
            out_nm = out_d.ap().rearrange("(p a) c -> p (a c)", p=P)

            # segment-0 idx load first, split across both HWDGE engines so
            # the first gathers start ASAP; later segments' idx tiles are
            # loaded after the first gather wave is issued.
            engs = [nc.sync, nc.scalar]
            kq = K // 2
            for j in range(2):
                engs[j].dma_start(
                    out=idx_sb[0][:, j * kq * ipg:(j + 1) * kq * ipg],
                    in_=idx_d.ap()[:, j * kq * ipg:(j + 1) * kq * ipg],
                )
            # shared register for the (constant) per-gather index count
            nreg = nc.gpsimd.to_reg(GATHER_CHUNK)

            # warmup: a tiny gather issued immediately absorbs the one-time
            # GPSIMD library/ring init stall while the idx loads run
            warm_idx = pool.tile([P, 1], mybir.dt.int16, tag="warmidx")
            warm_out = pool.tile([P, C], mybir.dt.float32, tag="warmout")
            nc.gpsimd.memset(warm_idx[:], 0)
            nc.gpsimd.dma_gather(
                out_ap=warm_out[:].rearrange("p (a c) -> p a c", c=C),
                in_ap=xt_d.ap(),
                idxs_ap=warm_idx[:],
                num_idxs=16,
                num_idxs_reg=nc.gpsimd.to_reg(16),
                elem_size=C,
                queue_num=3,
                single_packet=SINGLE_PACKET,
            )

            gi = 0
            for c in range(cpk):
                for k in range(K):
                    nc.gpsimd.dma_gather(
                        out_ap=g[k][c][:].rearrange("p (a c) -> p a c", c=C),
                        in_ap=xt_d.ap(),
                        idxs_ap=idx_sb[c][:, k * ipg:(k + 1) * ipg],
                        num_idxs=GATHER_CHUNK,
                        num_idxs_reg=nreg,
                        elem_size=C,
                        queue_num=gi % 4,
                        single_packet=SINGLE_PACKET,
                    )
                    gi += 1
                if c == 0:
                    # remaining idx segments + xt/alpha, issued after the
                    # first gather wave so they never gate gather 0
                    for cc in range(1, cpk):
                        nc.sync.dma_start(
                            out=idx_sb[cc][:],
                            in_=idx_d.ap()[:, cc * K * ipg:
                                           (cc + 1) * K * ipg],
                        )
                    nc.scalar.dma_start(out=al_sb[:], in_=alpha_d.ap())
                    nc.scalar.dma_start(out=xt_sb[:], in_=xt_nm)
                    nc.scalar.add(out=al_sb[:], in_=al_sb[:], add=1.0)

            for c in range(cpk):
                # o = xt*(1+alpha), then running accumulation in gather
                # completion (issue) order so only the last add is tail
                nc.vector.tensor_scalar_mul(
                    out=o[c][:], in0=xt_sb[:, c * SEGW:(c + 1) * SEGW],
                    scalar1=al_sb[:, :1],
                )
                for k in range(K):
                    nc.vector.tensor_add(
                        out=o[c][:], in0=o[c][:], in1=g[k][c][:],
                    )
                nc.sync.dma_start(
                    out=out_nm[:, c * SEGW:(c + 1) * SEGW], in_=o[c][:],
                )

    nc.compile()
    _split_multiwaits(nc, mybir)
    return nc


_PROGRAM = None


def _get_program():
    global _PROGRAM
    if _PROGRAM is None:
        _PROGRAM = _build_program()
    return _PROGRAM


# ---------------------------------------------------------------------------
# Host glue
# ---------------------------------------------------------------------------
_slot = np.arange(N)
_PERM = (_slot % P) * COLS + (_slot // P)  # node id for flat gather slot i


def _prep_idx(edge_b):
    """edge_b [N, K] int32 -> wrapped int16 [128, K*N/16] for dma_gather,
    laid out segment-major: col block (c, k) holds chunk c of gather k."""
    cpk = N // GATHER_CHUNK
    ipg = GATHER_CHUNK // 16
    ids = edge_b[_PERM, :].astype(np.int16)          # [4096 slots, K]
    f = ids.T.reshape(K, N // 16, 16)                # [K, s=256, p16]
    w = np.transpose(f, (2, 0, 1))                   # [p16, K, 256]
    w = np.tile(w, (8, 1, 1))                        # [128, K, 256]
    # wait: chunk c of gather k covers slots [c*CHUNK, (c+1)*CHUNK), i.e.
    # wrapped cols [c*ipg, (c+1)*ipg) of k's block -> reorder to (c, k, ipg)
    w = w.reshape(P, K, cpk, ipg).transpose(0, 2, 1, 3)
    return np.ascontiguousarray(w.reshape(P, K * (N // 16)))


def kernel(x, edge_index, alpha):
    global LAST_EXEC_NS
    _install_profile_shim()
    from concourse import bass_utils

    x = np.asarray(x)
    edge_index = np.asarray(edge_index)
    alpha_v = np.float32(np.asarray(alpha))

    nc = _get_program()

    xt = np.transpose(x[..., 0], (0, 2, 1))  # [B, N, C]
    in_maps = []
    for b in range(B):
        in_maps.append({
            "xt": np.ascontiguousarray(xt[b]),
            "idx": _prep_idx(edge_index[b]),
            "alpha": np.full((P, 1), alpha_v, dtype=np.float32),
        })

    trace = bool(int(os.environ.get("KERNEL_PROFILE", "0")))
    res = bass_utils.run_bass_kernel_spmd(
        nc, in_maps, core_ids=list(range(NCORES)), trace=trace
    )
    LAST_EXEC_NS = res.exec_time_ns

    out = np.empty((B, C, N, 1), dtype=np.float32)
    for b in range(B):
        out[b, :, :, 0] = res.results[b]["out"].T
    return out


# revision 29
# speedup vs baseline: 1.0093x; 1.0093x over previous
"""Trainium2 Bass kernel for nn_MessagePassingLayer (gnn_message_passing).

Computes, for x:[B,C,N,1] f32, edge_index:[B,N,K] i32, alpha scalar:
    out[b,c,n] = x[b,c,n]*(1+alpha) + sum_k x[b,c,edge_index[b,n,k]]

Sharding: B=8 batch samples, one per NeuronCore (data parallel). Edge
indices are intra-sample so there is no cross-core communication.

Per-core device program:
  - load node-major table xt [N=4096, C=64] (host-transposed layout)
  - 16x dma_gather (SWDGE row gather from HBM, 4096 rows x 256B each)
  - DVE pairwise-tree accumulation of the 16 gathered tiles
  - out = xt*(1+alpha) + sum  (DVE), stored node-major; host transposes back
"""
import os
import sys
import types

import numpy as np

B, C, N, K = 8, 64, 4096, 16
NCORES = 8
P = 128
COLS = N // P  # 32 nodes per partition
FREE = COLS * C  # 2048 f32 per partition

LAST_EXEC_NS = None


# ---------------------------------------------------------------------------
# axon NTFF profile hook shim (the agent image's antenv lacks axon_hooks)
# ---------------------------------------------------------------------------
def _install_profile_shim():
    if "antenv.axon_hooks" in sys.modules:
        return
    try:
        import antenv

        mod = types.ModuleType("antenv.axon_hooks")
        mod._hook = None
        mod.set_axon_ntff_profile_hook = lambda h: setattr(mod, "_hook", h)
        mod.get_axon_ntff_profile_hook = lambda: mod._hook
        sys.modules["antenv.axon_hooks"] = mod
        antenv.axon_hooks = mod
        from trn_agent_boot.trn_boot import _ntff_profile_via_ctypes

        mod.set_axon_ntff_profile_hook(
            _ntff_profile_via_ctypes("/opt/axon/libaxon_pjrt.so")
        )
    except Exception:
        pass


# ---------------------------------------------------------------------------
# Walrus in this container rejects >1 sync-wait per instruction. Split any
# multi-wait instruction into single-wait NoOps on the same engine.
# ---------------------------------------------------------------------------
def _split_multiwaits(nc, mybir):
    cnt = [0]
    for f in nc.m.functions:
        for bb in f.blocks:
            new_list = []
            for ins in bb.instructions:
                si = ins.sync_info
                if si is not None and si.on_wait and len(si.on_wait) > 1:
                    waits = list(si.on_wait)
                    for w in waits[:-1]:
                        cnt[0] += 1
                        nop = mybir.InstNoOp(name=f"I-waitsplit-{cnt[0]}")
                        nop.engine = ins.engine
                        nop.sync_info = mybir.SyncInfo(on_wait=[w], on_update=[])
                        try:
                            nc.register_instruction(nop, overwrite=True)
                        except Exception:
                            pass
                        new_list.append(nop)
                    ins.sync_info = mybir.SyncInfo(
                        on_wait=[waits[-1]], on_update=list(si.on_update)
                    )
                new_list.append(ins)
            bb.instructions = new_list


# ---------------------------------------------------------------------------
# Device program
# ---------------------------------------------------------------------------
GATHER_CHUNK = int(os.environ.get("KERNEL_GATHER_CHUNK", "2048"))
SCRATCH = int(os.environ.get("KERNEL_SCRATCH", "16384"))
SINGLE_PACKET = bool(int(os.environ.get("KERNEL_SINGLE_PACKET", "0")))


def _build_program():
    import concourse.mybir as mybir
    import concourse.tile as tile
    from concourse import bacc

    nc = bacc.Bacc("TRN2", target_bir_lowering=False, debug=False,
                   num_devices=NCORES, num_swdge_queues=4,
                   dynamic_dma_scratch_size=SCRATCH)
    xt_d = nc.dram_tensor("xt", [N, C], mybir.dt.float32, kind="ExternalInput")
    idx_d = nc.dram_tensor("idx", [P, K * (N // 16)], mybir.dt.int16,
                           kind="ExternalInput")
    alpha_d = nc.dram_tensor("alpha", [P, 1], mybir.dt.float32,
                             kind="ExternalInput")
    out_d = nc.dram_tensor("out", [N, C], mybir.dt.float32,
                           kind="ExternalOutput")

    cpk = N // GATHER_CHUNK      # segments (chunk instructions per k)
    ipg = GATHER_CHUNK // 16     # idx cols per chunk
    opg = GATHER_CHUNK // P      # out free-cols per chunk
    SEGW = opg * C               # f32 cols per segment in node-major tiles

    with tile.TileContext(nc) as tc:
        with tc.tile_pool(name="sbuf", bufs=1) as pool:
            xt_sb = pool.tile([P, FREE], mybir.dt.float32, tag="xt")
            idx_sb = [pool.tile([P, K * ipg], mybir.dt.int16, tag=f"idx{c}",
                                name=f"idx{c}") for c in range(cpk)]
            al_sb = pool.tile([P, 1], mybir.dt.float32, tag="al")
            g = [[pool.tile([P, SEGW], mybir.dt.float32, tag=f"g{k}_{c}",
                            name=f"g{k}_{c}") for c in range(cpk)]
                 for k in range(K)]
            o = [pool.tile([P, SEGW], mybir.dt.float32, tag=f"o{c}",
                           name=f"o{c}") for c in range(cpk)]

            xt_nm = xt_d.ap().rearrange("(p a) c -> p (a c)", p=P)
            out_nm = out_d.ap().rearrange("(p a) c -> p (a c)", p=P)

            # segment-0 idx load first, split across both HWDGE engines so
            # the first gathers start ASAP; later segments' idx tiles are
            # loaded after the first gather wave is issued.
            engs = [nc.sync, nc.scalar]
            kq = K // 2
            for j in range(2):
                engs[j].dma_start(
                    out=idx_sb[0][:, j * kq * ipg:(j + 1) * kq * ipg],
                    in_=idx_d.ap()[:, j * kq * ipg:(j + 1) * kq * ipg],
                )
            # shared register for the (constant) per-gather index count
            nreg = nc.gpsimd.to_reg(GATHER_CHUNK)

            gi = 0
            for c in range(cpk):
                for k in range(K):
                    nc.gpsimd.dma_gather(
                        out_ap=g[k][c][:].rearrange("p (a c) -> p a c", c=C),
                        in_ap=xt_d.ap(),
                        idxs_ap=idx_sb[c][:, k * ipg:(k + 1) * ipg],
                        num_idxs=GATHER_CHUNK,
                        num_idxs_reg=nreg,
                        elem_size=C,
                        queue_num=gi % 4,
                        single_packet=SINGLE_PACKET,
                    )
                    gi += 1
                if c == 0:
                    # remaining idx segments + xt/alpha, issued after the
                    # first gather wave so they never gate gather 0
                    for cc in range(1, cpk):
                        nc.sync.dma_start(
                            out=idx_sb[cc][:],
                            in_=idx_d.ap()[:, cc * K * ipg:
                                           (cc + 1) * K * ipg],
                        )
                    nc.scalar.dma_start(out=al_sb[:], in_=alpha_d.ap())
                    nc.scalar.dma_start(out=xt_sb[:], in_=xt_nm)
                    nc.scalar.add(out=al_sb[:], in_=al_sb[:], add=1.0)

            for c in range(cpk):
                # o = xt*(1+alpha), then running accumulation in gather
                # completion (issue) order so only the last add is tail
                nc.vector.tensor_scalar_mul(
                    out=o[c][:], in0=xt_sb[:, c * SEGW:(c + 1) * SEGW],
                    scalar1=al_sb[:, :1],
                )
                for k in range(K):
                    nc.vector.tensor_add(
                        out=o[c][:], in0=o[c][:], in1=g[k][c][:],
                    )
                nc.sync.dma_start(
                    out=out_nm[:, c * SEGW:(c + 1) * SEGW], in_=o[c][:],
                )

    nc.compile()
    _split_multiwaits(nc, mybir)
    return nc


_PROGRAM = None


def _get_program():
    global _PROGRAM
    if _PROGRAM is None:
        _PROGRAM = _build_program()
    return _PROGRAM


# ---------------------------------------------------------------------------
# Host glue
# ---------------------------------------------------------------------------
_slot = np.arange(N)
_PERM = (_slot % P) * COLS + (_slot // P)  # node id for flat gather slot i


def _prep_idx(edge_b):
    """edge_b [N, K] int32 -> wrapped int16 [128, K*N/16] for dma_gather,
    laid out segment-major: col block (c, k) holds chunk c of gather k."""
    cpk = N // GATHER_CHUNK
    ipg = GATHER_CHUNK // 16
    ids = edge_b[_PERM, :].astype(np.int16)          # [4096 slots, K]
    f = ids.T.reshape(K, N // 16, 16)                # [K, s=256, p16]
    w = np.transpose(f, (2, 0, 1))                   # [p16, K, 256]
    w = np.tile(w, (8, 1, 1))                        # [128, K, 256]
    # wait: chunk c of gather k covers slots [c*CHUNK, (c+1)*CHUNK), i.e.
    # wrapped cols [c*ipg, (c+1)*ipg) of k's block -> reorder to (c, k, ipg)
    w = w.reshape(P, K, cpk, ipg).transpose(0, 2, 1, 3)
    return np.ascontiguousarray(w.reshape(P, K * (N // 16)))


def kernel(x, edge_index, alpha):
    global LAST_EXEC_NS
    _install_profile_shim()
    from concourse import bass_utils

    x = np.asarray(x)
    edge_index = np.asarray(edge_index)
    alpha_v = np.float32(np.asarray(alpha))

    nc = _get_program()

    xt = np.transpose(x[..., 0], (0, 2, 1))  # [B, N, C]
    in_maps = []
    for b in range(B):
        in_maps.append({
            "xt": np.ascontiguousarray(xt[b]),
            "idx": _prep_idx(edge_index[b]),
            "alpha": np.full((P, 1), alpha_v, dtype=np.float32),
        })

    trace = bool(int(os.environ.get("KERNEL_PROFILE", "0")))
    res = bass_utils.run_bass_kernel_spmd(
        nc, in_maps, core_ids=list(range(NCORES)), trace=trace
    )
    LAST_EXEC_NS = res.exec_time_ns

    out = np.empty((B, C, N, 1), dtype=np.float32)
    for b in range(B):
        out[b, :, :, 0] = res.results[b]["out"].T
    return out
